# revision 1
# baseline (speedup 1.0000x reference)
"""FKANLinear fused kernel for 8 TRN2 NeuronCores.

Strategy (data-parallel over batch):
  y[b,o] = alpha*(x@W'.T + bias)[b,o] + beta*sum_f einsum('bik,oik->bo', Phi_f, Ceff_f)

All per-(o,i,k) coefficient tensors are folded on the host into per-plane
coefficients of 31 feature planes of x:
  planes = {x, x^2, x^3, x^4} u {cos(kx), sin(kx)}_{k=1..8} u {wavelet_c}_{c=0..3}
           u {relu(x-kappa_j)^3}_{j=1..7}
The polynomial families (taylor/jacobi/cheby + the bspline's polynomial part)
fold onto the monomials; the bspline is rewritten in a truncated-power basis in
normalized coordinates t = (x-xmin)*8/(xmax-xmin), whose basis change is static.
Only the affine map x->t is data dependent (global per-column min/max), handled
on-device with a tiny AllGather and an on-device coefficient remix.

Device: per 512-column block: PE accumulates 16 chunk matmuls (contraction over
(plane,i) pairs, 128 rows per chunk) into PSUM; ACT builds sin/exp/square
planes; DVE/GPSIMD build the rest.
"""

import sys
import numpy as np

if "/opt/trn_rl_repo" not in sys.path:
    sys.path.insert(0, "/opt/trn_rl_repo")

N_CORES = 8
B, IN, OUT = 32768, 64, 32
BS = B // N_CORES          # 4096 rows per core
BC = 512                   # batch columns per block
NBCOL = BS // BC           # 8
NTILES = BS // 128         # 32 x-tiles per core
G, P = 8, 3
TAY = 4
JDEG, JA, JB = 4, 1.0, 1.0
CDEG = 4
FREQ = 8
WCH = 4
TEMP = 2.0

F32 = np.float32


# ----------------------------------------------------------------------------
# host-side folding
# ----------------------------------------------------------------------------

def _softplus(z):
    z = np.asarray(z, np.float64)
    return np.log1p(np.exp(-np.abs(z))) + np.maximum(z, 0.0)


def _softmax(z, axis):
    z = np.asarray(z, np.float64)
    m = z.max(axis=axis, keepdims=True)
    e = np.exp(z - m)
    return e / e.sum(axis=axis, keepdims=True)


def _jacobi_mono():
    """Monomial coefficients of the reference's jacobi_bases terms, (JDEG+1, 5)."""
    a, b = JA, JB
    terms = np.zeros((JDEG + 1, 5))
    terms[0, 0] = 1.0
    if JDEG >= 1:
        # 0.5*(2(a+1)x + (a-b))/sqrt(2)
        terms[1, 1] = 0.5 * 2.0 * (a + 1.0) / np.sqrt(2.0)
        terms[1, 0] = 0.5 * (a - b) / np.sqrt(2.0)
    for n in range(2, JDEG + 1):
        k = n - 1
        A1 = 2 * k + a + b
        A2 = 2 * (k + 1) * (k + a + b + 1) * (A1 + 1)
        A4 = 2 * (k + a) * (k + b) * (A1 + 2)
        # Jn = (A1+1)*((A1+2)*A1*x + a^2-b^2)/A2 * terms[n-1] - A4/A2*terms[n-2]
        c_x = (A1 + 1) * (A1 + 2) * A1 / A2
        c_0 = (A1 + 1) * (a * a - b * b) / A2
        Jn = np.zeros(5)
        Jn[1:] += c_x * terms[n - 1][:4]
        Jn += c_0 * terms[n - 1]
        Jn -= (A4 / A2) * terms[n - 2]
        terms[n] = Jn / np.sqrt(n + 1.0)
    return terms


def _cheby_mono():
    """Monomial coefficients of cheby_bases rows (CDEG+1, 5) incl /sqrt(n+1)."""
    T = np.zeros((CDEG + 1, 5))
    T[0, 0] = 1.0
    T[1, 1] = 1.0
    for n in range(2, CDEG + 1):
        T[n] = 2.0 * np.roll(T[n - 1], 1) - T[n - 2]
        T[n][0] = -T[n - 2][0]  # roll wraps; x*T has no constant from T[n-1][4]
    # recompute safely: x*poly shift without wraparound
    T = np.zeros((CDEG + 1, 5))
    T[0, 0] = 1.0
    T[1, 1] = 1.0
    for n in range(2, CDEG + 1):
        shift = np.zeros(5)
        shift[1:] = T[n - 1][:4]
        T[n] = 2.0 * shift - T[n - 2]
    norm = 1.0 / np.sqrt(np.arange(CDEG + 1) + 1.0)
    return T * norm[:, None]


def _bspline_tspace_phi(t):
    """Reference Cox-de Boor bases evaluated in normalized coords t in [0,8]."""
    grid = np.concatenate([np.zeros(3), np.linspace(0.0, 8.0, G + 1), np.full(3, 8.0)])
    te = t[:, None]
    bases = ((te >= grid[None, :-1]) & (te < grid[None, 1:])).astype(np.float64)
    mask_last = t == grid[-1]
    bases[mask_last, :] = 0.0
    bases[mask_last, -1] = 1.0
    for r in range(1, P + 1):
        ld = np.maximum(grid[r:-1] - grid[:-(r + 1)], 1e-12)
        rd = np.maximum(grid[r + 1:] - grid[1:-r], 1e-12)
        left = (te - grid[None, :-(r + 1)]) / ld[None, :] * bases[:, :-1]
        right = (grid[None, r + 1:] - te) / rd[None, :] * bases[:, 1:]
        bases = left + right
    return bases  # (S, G+P)


def _bspline_truncpow_matrix():
    """M (11, 11): B_k(t) = sum_m M[m, k] * F_m(t),
    F = [1, t, t^2, t^3, relu(t-1)^3 .. relu(t-7)^3]."""
    S = 6000
    t = np.linspace(0.0, 8.0, S)
    # avoid exact interior knots for the fit (rep is continuous anyway)
    t = t + 1e-7
    t = np.clip(t, 0.0, 8.0)
    phi = _bspline_tspace_phi(t)  # (S, 11)
    Fm = np.zeros((S, 11))
    Fm[:, 0] = 1.0
    Fm[:, 1] = t
    Fm[:, 2] = t * t
    Fm[:, 3] = t ** 3
    for j in range(1, 8):
        Fm[:, 3 + j] = np.maximum(t - j, 0.0) ** 3
    M, res, rank, _ = np.linalg.lstsq(Fm, phi, rcond=None)
    return M  # (11 features, 11 bases)


def fold_constants(inputs):
    """Returns dict of small f32 arrays for the device."""
    x = inputs["x"]
    base_v = np.asarray(inputs["base_v"], np.float64)
    base_g = np.asarray(inputs["base_g"], np.float64)
    base_bias = np.asarray(inputs["base_bias"], np.float64)
    gains = np.asarray(inputs["gains"], np.float64)
    alpha = float(_softplus(inputs["alpha_logit"]))
    beta = float(_softplus(inputs["beta_logit"]))
    mixw = _softmax(np.asarray(inputs["mix_logits"], np.float64) / TEMP, axis=-1)
    sg = _softplus(gains)

    def ceff(name, f):
        return np.asarray(inputs[name], np.float64) * mixw[..., f:f + 1] * sg[f] * beta

    C_bs = ceff("bspline_coef", 0)
    C_ty = ceff("taylor_coef", 1)
    C_jb = ceff("jacobi_coef", 2)
    C_cb = ceff("cheby_coef", 3)
    C_fr = ceff("fourier_coef", 4)
    C_wv = ceff("wavelet_coef", 5)

    # base linear
    vn = np.sqrt((base_v ** 2).sum(axis=1, keepdims=True))
    Walpha = alpha * base_g * base_v / vn              # (32, 64)
    bias_alpha = alpha * base_bias                      # (32,)

    # static monomial folds (degree 0..4)
    mono = np.zeros((OUT, IN, 5))
    fac = np.array([1.0, 1.0, 2.0, 6.0])
    mono[:, :, :4] += C_ty / fac[None, None, :]
    Jc = _jacobi_mono()
    mono += np.einsum("oin,nd->oid", C_jb, Jc)
    Cc = _cheby_mono()
    mono += np.einsum("oin,nd->oid", C_cb, Cc)

    # fourier: cos(kx) = T_k(cos x), sin(kx) = sin(x) * U_{k-1}(cos x).
    # Planes are (c^j | s*c^(j-1)) for j=1..8; Chebyshev coefficient change is
    # folded here (T_k const terms go to the ones-plane / CO).
    fnorm = 1.0 / np.sqrt(2.0 * FREQ)
    Ccos = C_fr[:, :, :FREQ] * fnorm                    # (32,64,8) coef of cos(kx)
    Csin = C_fr[:, :, FREQ:] * fnorm                    # coef of sin(kx)
    # T_k monomial coeffs, k=1..8 (degree up to 8); U_{k-1} for k=1..8 (deg k-1)
    Tc = np.zeros((9, 9)); Tc[0, 0] = 1.0; Tc[1, 1] = 1.0
    Uc = np.zeros((9, 9)); Uc[0, 0] = 1.0; Uc[1, 1] = 2.0
    for n in range(2, 9):
        for M_ in (Tc, Uc):
            sh = np.zeros(9); sh[1:] = M_[n - 1][:8]
            M_[n] = 2.0 * sh - M_[n - 2]
    # c-power planes coefficients (j=0..8) and s*c-power (j=0..7)
    Ccpow = np.einsum("oik,kj->oij", Ccos, Tc[1:9, :])   # (32,64,9): j=0..8
    Cspow = np.einsum("oik,kj->oij", Csin, Uc[0:8, :])   # (32,64,9): j=0..8 (j=8 zero)

    # wavelet
    a_w = _softplus(np.asarray(inputs["wavelet_scale_logit"], np.float64)) + 1e-6
    inva = 1.0 / a_w                                     # (64,4)
    nshia = -np.asarray(inputs["wavelet_shift"], np.float64) * inva

    # bspline -> truncated powers in t
    M = _bspline_truncpow_matrix()                       # (11 feat, 11 bases)
    # Ceff_bspline combined with basis change: coef over features
    CF = np.einsum("oik,mk->oim", C_bs, M)               # (32,64,11): m: 0..3 t^d, 4..10 rho_j
    P_poly = CF[:, :, :4]                                # (32,64,4)
    R_rho = CF[:, :, 4:]                                 # (32,64,7)

    # CW: (128, 16*32): chunk c columns 32c..32c+32
    CW = np.zeros((128, 16 * OUT))
    def put(c, half, arr_oi):
        # arr (32, 64) -> rows half*64..+64 (i), col 32c+o
        CW[half * 64:(half + 1) * 64, 32 * c:32 * (c + 1)] = arr_oi.T
    put(0, 0, Walpha + mono[:, :, 1])
    put(0, 1, mono[:, :, 2])
    put(1, 0, mono[:, :, 3])
    put(1, 1, mono[:, :, 4])
    for j in range(1, 9):
        put(1 + j, 0, Ccpow[:, :, j])        # c^j plane
        put(1 + j, 1, Cspow[:, :, j - 1])    # s*c^(j-1) plane
    put(10, 0, C_wv[:, :, 0]); put(10, 1, C_wv[:, :, 1])
    put(11, 0, C_wv[:, :, 2]); put(11, 1, C_wv[:, :, 3])
    # c12..15 rho chunks filled on device

    CO = (mono[:, :, 0] + Ccpow[:, :, 0]).T.copy()       # (64, 32)
    CO[0, :] += bias_alpha

    # device vector chains run 128-partition wide (rows 64:128 duplicate 0:64)
    PW = np.zeros((128, 4 * OUT))
    for d in range(4):
        PW[0:64, 32 * d:32 * (d + 1)] = P_poly[:, :, d].T
    PW[64:128] = PW[0:64]
    RW = np.zeros((128, 7 * OUT))
    for j in range(7):
        RW[0:64, 32 * j:32 * (j + 1)] = R_rho[:, :, j].T
    RW[64:128] = RW[0:64]

    WVP = np.zeros((128, 4))
    for p in range(2):
        WVP[0:64, 2 * p] = inva[:, 2 * p]
        WVP[64:128, 2 * p] = inva[:, 2 * p + 1]
        WVP[0:64, 2 * p + 1] = nshia[:, 2 * p]
        WVP[64:128, 2 * p + 1] = nshia[:, 2 * p + 1]

    FB = np.zeros((128, 1))
    FB[0:64, 0] = np.pi / 2.0

    ID = np.eye(128)

    return {
        "CW": CW.astype(F32), "CO": CO.astype(F32), "PW": PW.astype(F32),
        "RW": RW.astype(F32), "WVP": WVP.astype(F32), "FB": FB.astype(F32),
        "ID": ID.astype(F32),
    }


# ----------------------------------------------------------------------------
# numpy emulation of the device algorithm (for validating the folding)
# ----------------------------------------------------------------------------

def numpy_forward(inputs):
    consts = fold_constants(inputs)
    x = np.asarray(inputs["x"], np.float64)  # (B, 64)
    CW = consts["CW"].astype(np.float64)
    CO = consts["CO"].astype(np.float64)
    PW = consts["PW"].astype(np.float64)
    RW = consts["RW"].astype(np.float64)
    WVP = consts["WVP"].astype(np.float64)

    xmin = x.min(axis=0); xmax = x.max(axis=0)
    pad = (xmax - xmin) < 1e-8
    xmin = np.where(pad, xmin - 0.5, xmin)
    xmax = np.where(pad, xmax + 0.5, xmax)
    rng = xmax - xmin
    b = 8.0 / rng
    a = -xmin * b
    # dynamic poly remix: t^d = (a + b x)^d -> x^e coefficients
    P_poly = np.stack([PW[0:64, 32 * d:32 * (d + 1)] for d in range(4)], axis=-1)  # (64,32,4)
    binom = {(0, 0): 1, (1, 0): 1, (1, 1): 1, (2, 0): 1, (2, 1): 2, (2, 2): 1,
             (3, 0): 1, (3, 1): 3, (3, 2): 3, (3, 3): 1}
    Cdyn = np.zeros((IN, OUT, 4))
    for d in range(4):
        for e in range(d + 1):
            Cdyn[:, :, e] += P_poly[:, :, d] * (binom[(d, e)] * a ** (d - e) * b ** e)[:, None]
    CW = CW.copy()
    CW[0:64, 0:32] += Cdyn[:, :, 1]
    CW[64:128, 0:32] += Cdyn[:, :, 2]
    CW[0:64, 32:64] += Cdyn[:, :, 3]
    CO = CO + Cdyn[:, :, 0]
    bias = CO.sum(axis=0)  # (32,)
    # rho coefficients
    for j in range(1, 8):
        c = 12 + (j - 1) // 2
        half = (j - 1) % 2
        CW[half * 64:(half + 1) * 64, 32 * c:32 * (c + 1)] = \
            RW[0:64, 32 * (j - 1):32 * j] * (b ** 3)[:, None]

    # features
    Bn = x.shape[0]
    y = np.tile(bias[None, :], (Bn, 1))
    planes = np.zeros((Bn, 128))
    kap = [xmin + j * rng / 8.0 for j in range(1, 8)]

    def chunk_feat(c):
        f = np.zeros((Bn, 128))
        if c == 0:
            f[:, 0:64] = x; f[:, 64:128] = x * x
        elif c == 1:
            f[:, 0:64] = x ** 3; f[:, 64:128] = x ** 4
        elif 2 <= c <= 9:
            j = c - 1
            cc, ss = np.cos(x), np.sin(x)
            f[:, 0:64] = cc ** j; f[:, 64:128] = ss * cc ** (j - 1)
        elif c in (10, 11):
            p = c - 10
            u0 = x * WVP[None, 0:64, 2 * p] + WVP[None, 0:64, 2 * p + 1]
            u1 = x * WVP[None, 64:128, 2 * p] + WVP[None, 64:128, 2 * p + 1]
            f[:, 0:64] = (u0 ** 2 - 1) * np.exp(-0.5 * u0 ** 2)
            f[:, 64:128] = (u1 ** 2 - 1) * np.exp(-0.5 * u1 ** 2)
        else:
            j0 = 2 * (c - 12) + 1
            f[:, 0:64] = np.maximum(x - kap[j0 - 1][None, :], 0.0) ** 3
            if j0 + 1 <= 7:
                f[:, 64:128] = np.maximum(x - kap[j0][None, :], 0.0) ** 3
        return f

    for c in range(16):
        f = chunk_feat(c)
        rows = 64 if c == 15 else 128
        y = y + f[:, :rows] @ CW[:rows, 32 * c:32 * (c + 1)]
    return y.astype(F32)


# ----------------------------------------------------------------------------
# device kernel
# ----------------------------------------------------------------------------

def build_nc(debug=False, reps=1, no_collective=False):
    import concourse.bass as bass
    import concourse.bacc as bacc
    import concourse.mybir as mybir
    import concourse.tile as tile

    dt = mybir.dt.float32
    dtr = mybir.dt.float32r
    AF = mybir.ActivationFunctionType
    ALU = mybir.AluOpType
    AX = mybir.AxisListType

    SBC = 1024                  # elementwise super-block columns
    NSUP = BS // SBC            # 4
    GRP = 512                   # rows per batched x-load DMA (4 tiles)

    def f32r(ap):
        return ap if ap.dtype == mybir.dt.float32r else ap.bitcast(mybir.dt.float32r)

    def dup64(ap_slice):
        """(128, 64) view -> (128, 2, 64) zero-stride duplicated view."""
        lst = list(ap_slice.ap)
        new = [list(lst[0]), [0, 2]] + [list(d) for d in lst[1:]]
        return ap_slice.__replace__(ap=new)

    nc = bacc.Bacc("TRN2", target_bir_lowering=False, debug=False,
                   enable_asserts=True, num_devices=N_CORES)

    xs = nc.dram_tensor("xs", [BS, IN], dt, kind="ExternalInput").ap()
    cw_d = nc.dram_tensor("CW", [128, 16 * OUT], dtr, kind="ExternalInput").ap()
    co_d = nc.dram_tensor("CO", [IN, OUT], dt, kind="ExternalInput").ap()
    pw_d = nc.dram_tensor("PW", [128, 4 * OUT], dt, kind="ExternalInput").ap()
    rw_d = nc.dram_tensor("RW", [128, 7 * OUT], dt, kind="ExternalInput").ap()
    wv_d = nc.dram_tensor("WVP", [128, 4], dt, kind="ExternalInput").ap()
    fb_d = nc.dram_tensor("FB", [128, 1], dt, kind="ExternalInput").ap()
    id_d = nc.dram_tensor("ID", [128, 128], dt, kind="ExternalInput").ap()
    # output is transposed (OUT, BS); host does the final transpose
    y_d = nc.dram_tensor("y", [OUT, BS], dt, kind="ExternalOutput").ap()
    dbg = {}
    if debug:
        for nm, shape in [("d_xdup", [128, 256]), ("d_locmin", [128, 1]),
                          ("d_locmax", [128, 1]), ("d_gm", [128, 16]),
                          ("d_vecs", [128, 24]), ("d_kn", [128, 7]),
                          ("d_kp", [128, 4]), ("d_cw", [128, 512]),
                          ("d_biasv", [32, 1]), ("d_y1", [32, 512]),
                          ("d_m0", [128, 256]), ("d_m1", [128, 256]),
                          ("d_wf", [128, 256]), ("d_r3", [128, 256]),
                          ("d_four", [128, 256])]:
            dbg[nm] = nc.dram_tensor(nm, shape, dt, kind="ExternalOutput").ap()

    with tile.TileContext(nc) as tc:
        with (
            tc.tile_pool(name="const", bufs=1) as cpool,
            tc.tile_pool(name="sb", bufs=1) as sb,
            tc.tile_pool(name="fourp", bufs=2) as fourp,
            tc.tile_pool(name="xpipe", bufs=3) as xpipe,
            tc.tile_pool(name="pers", bufs=1) as pers,
            tc.tile_pool(name="ps", bufs=2, space="PSUM") as ps,
            tc.tile_pool(name="psacc", bufs=2, space="PSUM") as psacc,
            tc.tile_pool(name="dram", bufs=1, space="DRAM") as dram,
        ):
            # constants
            cwt = cpool.tile([128, 16 * OUT], dtr, tag="cwt")
            cot = cpool.tile([IN, OUT], dt, tag="cot")
            pwt = cpool.tile([128, 4 * OUT], dt, tag="pwt")
            rwt = cpool.tile([128, 7 * OUT], dt, tag="rwt")
            wvt = cpool.tile([128, 4], dt, tag="wvt")
            fbt = cpool.tile([128, 1], dt, tag="fbt")
            idt = cpool.tile([128, 128], dt, tag="idt")
            nc.sync.dma_start(out=cwt[:, :], in_=cw_d[:, :])
            nc.sync.dma_start(out=cot[:, :], in_=co_d[:, :])
            nc.sync.dma_start(out=pwt[:, :], in_=pw_d[:, :])
            nc.sync.dma_start(out=rwt[:, :], in_=rw_d[:, :])
            nc.sync.dma_start(out=wvt[:, :], in_=wv_d[:, :])
            nc.sync.dma_start(out=fbt[:, :], in_=fb_d[:, :])
            nc.sync.dma_start(out=idt[:, :], in_=id_d[:, :])

            ones64 = pers.tile([IN, 1], dt, tag="ones64")
            nc.vector.memset(ones64[:, :], 1.0)

            # repetition loop used only by the timing harness (reps>1):
            # outputs of reps>1 are numerically bogus (CW re-remixed), but
            # the instruction stream per rep is identical.
            for _rep in range(reps):
                mm = pers.tile([128, 2 * NSUP], dt, tag="mm")
                xds = []

                # ------------ phase A: load, transpose, min/max ------------
                for s in range(NSUP):
                    xd = pers.tile([128, SBC], dt, tag=f"xd{s}")
                    xds.append(xd)
                    for g in range(SBC // GRP):          # 2 load-groups
                        base = s * SBC + g * GRP
                        # interleaved double-load: [t0 t0 t1 t1 t2 t2 t3 t3]
                        # so each transpose reads a contiguous [x|x] block
                        xt8 = xpipe.tile([128, 8 * IN], dt, tag="xin")
                        src = xs[base:base + GRP, :].rearrange(
                            "(t p) i -> p t i", p=128)
                        dst = xt8.rearrange("p (t i) -> p t i", i=2 * IN)
                        nc.sync.dma_start(out=dst[:, :, 0:IN], in_=src)
                        nc.sync.dma_start(out=dst[:, :, IN:2 * IN], in_=src)
                        tp = ps.tile([128, 512], dt, tag="tp")
                        for t in range(4):
                            nc.tensor.transpose(
                                tp[:, 128 * t:128 * (t + 1)],
                                xt8[:, t * 128:(t + 1) * 128], idt[:, :])
                        nc.scalar.copy(xd[:, g * GRP:(g + 1) * GRP], tp[:, :])
                    nc.vector.tensor_reduce(out=mm[:, s:s + 1], in_=xd[:, :],
                                            axis=AX.X, op=ALU.min)
                    nc.vector.tensor_reduce(out=mm[:, NSUP + s:NSUP + s + 1],
                                            in_=xd[:, :], axis=AX.X, op=ALU.max)

                locmin = pers.tile([128, 1], dt, tag="locmin")
                locmax = pers.tile([128, 1], dt, tag="locmax")
                nc.vector.tensor_reduce(out=locmin[:, :], in_=mm[:, 0:NSUP],
                                        axis=AX.X, op=ALU.min)
                nc.vector.tensor_reduce(out=locmax[:, :], in_=mm[:, NSUP:2 * NSUP],
                                        axis=AX.X, op=ALU.max)
                if debug:
                    nc.sync.dma_start(out=dbg["d_xdup"][:, :], in_=xds[0][:, 0:256])
                    nc.sync.dma_start(out=dbg["d_locmin"][:, :], in_=locmin[:, :])
                    nc.sync.dma_start(out=dbg["d_locmax"][:, :], in_=locmax[:, :])

                # ------------ collective: allgather min/max ------------
                bounce_in = dram.tile([2, IN], dt, tag="cin")
                bounce_out = dram.tile([2 * N_CORES, IN], dt, tag="cout")
                nc.sync.dma_start(out=bounce_in[0:1, :], in_=locmin[0:IN, :])
                nc.sync.dma_start(out=bounce_in[1:2, :], in_=locmax[0:IN, :])
                if no_collective:
                    for r in range(N_CORES):
                        nc.gpsimd.dma_start(out=bounce_out[2 * r:2 * r + 2, :],
                                            in_=bounce_in[:, :])
                else:
                    nc.gpsimd.collective_compute(
                        "AllGather", mybir.AluOpType.bypass,
                        replica_groups=[list(range(N_CORES))],
                        ins=[bounce_in.opt()],
                        outs=[bounce_out.opt()],
                    )
                gm = pers.tile([128, 2 * N_CORES], dt, tag="gm")
                nc.sync.dma_start(out=gm[0:IN, :],
                                  in_=bounce_out.rearrange("a b -> b a"))
                nc.sync.dma_start(out=gm[IN:128, :],
                                  in_=bounce_out.rearrange("a b -> b a"))

                # ------------ phase B: fourier chunks + round-1 matmuls -----
                y1s = []
                for s in range(NSUP):
                    xsl = xds[s][:, :]
                    sh = sb.tile([128, SBC], dt, tag="sh")
                    nc.scalar.activation(sh[:, :], xsl, AF.Sin, scale=0.5)
                    sq2 = sb.tile([128, SBC], dt, tag="sq2")
                    nc.scalar.square(sq2[:, :], sh[:, :])
                    sh4 = sb.tile([128, SBC], dt, tag="sh4")
                    nc.scalar.activation(sh4[:, :], xsl, AF.Sin, scale=0.25)
                    sq4 = sb.tile([128, SBC], dt, tag="sq4")
                    nc.scalar.square(sq4[:, :], sh4[:, :])
                    cdup = fourp.tile([128, SBC], dt, tag="cdup")
                    nc.vector.tensor_scalar(out=cdup[:, :], in0=sq2[:, :],
                                            scalar1=-2.0, scalar2=1.0,
                                            op0=ALU.mult, op1=ALU.add)
                    ch2 = sb.tile([128, SBC], dt, tag="ch2")
                    nc.vector.tensor_scalar(out=ch2[:, :], in0=sq4[:, :],
                                            scalar1=-2.0, scalar2=1.0,
                                            op0=ALU.mult, op1=ALU.add)
                    pk = fourp.tile([128, SBC], dtr, tag="p1")
                    nc.vector.tensor_copy(out=pk[0:IN, :], in_=cdup[0:IN, :])
                    nc.vector.scalar_tensor_tensor(
                        out=pk[IN:128, :], in0=sh[IN:128, :], scalar=2.0,
                        in1=ch2[IN:128, :], op0=ALU.mult, op1=ALU.mult)
                    if debug and s == 0:
                        nc.sync.dma_start(out=dbg["d_four"][:, :], in_=pk[:, 0:256].bitcast(dt))
                    acc_a = psacc.tile([OUT, BC], dt, tag="acc1")
                    acc_b = psacc.tile([OUT, BC], dt, tag="acc1")
                    nc.tensor.matmul(acc_a[:, :], f32r(cwt[:, 64:96]),
                                     f32r(pk[:, 0:BC]), start=True, stop=False)
                    nc.tensor.matmul(acc_b[:, :], f32r(cwt[:, 64:96]),
                                     f32r(pk[:, BC:SBC]), start=True, stop=False)
                    for j in range(2, 9):
                        pn = fourp.tile([128, SBC], dtr, tag=f"p{j}")
                        eng = nc.gpsimd if j in (3, 5, 7) else nc.vector
                        eng.tensor_tensor(out=pn[:, :], in0=pk[:, :],
                                          in1=cdup[:, :], op=ALU.mult)
                        lhs = f32r(cwt[:, 32 * (1 + j):32 * (2 + j)])
                        nc.tensor.matmul(acc_a[:, :], lhs, f32r(pn[:, 0:BC]),
                                         start=False, stop=(j == 8))
                        nc.tensor.matmul(acc_b[:, :], lhs, f32r(pn[:, BC:SBC]),
                                         start=False, stop=(j == 8))
                        pk = pn
                    y1 = pers.tile([OUT, SBC], dt, tag=f"y1_{s}")
                    nc.vector.tensor_copy(out=y1[:, 0:BC], in_=acc_a[:, :])
                    nc.vector.tensor_copy(out=y1[:, BC:SBC], in_=acc_b[:, :])
                    y1s.append(y1)
                    if debug and s == 0:
                        nc.sync.dma_start(out=dbg["d_y1"][:, :], in_=y1[:, 0:BC])

                # ------------ post-collective vector math ------------
                v = pers.tile([128, 24], dt, tag="vecs")
                gmin, gmax, rng_, msk = v[:, 0:1], v[:, 1:2], v[:, 2:3], v[:, 3:4]
                gmin2, gmax2, rng2 = v[:, 4:5], v[:, 5:6], v[:, 6:7]
                rinv, bb, aa = v[:, 7:8], v[:, 8:9], v[:, 9:10]
                b2, b3, a2, a3 = v[:, 10:11], v[:, 11:12], v[:, 12:13], v[:, 13:14]
                ab, a2b, ab2, rstep = (v[:, 14:15], v[:, 15:16], v[:, 16:17],
                                       v[:, 17:18])
                gmr = gm.rearrange("p (r t) -> p t r", t=2)
                nc.vector.tensor_reduce(out=gmin[:, :], in_=gmr[:, 0, :],
                                        axis=AX.X, op=ALU.min)
                nc.vector.tensor_reduce(out=gmax[:, :], in_=gmr[:, 1, :],
                                        axis=AX.X, op=ALU.max)
                nc.vector.tensor_tensor(out=rng_[:, :], in0=gmax[:, :],
                                        in1=gmin[:, :], op=ALU.subtract)
                nc.vector.tensor_scalar(out=msk[:, :], in0=rng_[:, :],
                                        scalar1=1e-8, scalar2=0.5,
                                        op0=ALU.is_lt, op1=ALU.mult)
                nc.vector.tensor_tensor(out=gmin2[:, :], in0=gmin[:, :],
                                        in1=msk[:, :], op=ALU.subtract)
                nc.vector.tensor_tensor(out=gmax2[:, :], in0=gmax[:, :],
                                        in1=msk[:, :], op=ALU.add)
                nc.vector.tensor_tensor(out=rng2[:, :], in0=gmax2[:, :],
                                        in1=gmin2[:, :], op=ALU.subtract)
                nc.vector.reciprocal(out=rinv[:, :], in_=rng2[:, :])
                nc.vector.tensor_scalar_mul(out=bb[:, :], in0=rinv[:, :],
                                            scalar1=8.0)
                nc.vector.scalar_tensor_tensor(out=aa[:, :], in0=gmin2[:, :],
                                               scalar=-1.0, in1=bb[:, :],
                                               op0=ALU.mult, op1=ALU.mult)
                nc.vector.tensor_tensor(out=b2[:, :], in0=bb[:, :], in1=bb[:, :],
                                        op=ALU.mult)
                nc.vector.tensor_tensor(out=b3[:, :], in0=b2[:, :], in1=bb[:, :],
                                        op=ALU.mult)
                nc.vector.tensor_tensor(out=a2[:, :], in0=aa[:, :], in1=aa[:, :],
                                        op=ALU.mult)
                nc.vector.tensor_tensor(out=a3[:, :], in0=a2[:, :], in1=aa[:, :],
                                        op=ALU.mult)
                nc.vector.tensor_tensor(out=ab[:, :], in0=aa[:, :], in1=bb[:, :],
                                        op=ALU.mult)
                nc.vector.tensor_tensor(out=a2b[:, :], in0=a2[:, :], in1=bb[:, :],
                                        op=ALU.mult)
                nc.vector.tensor_tensor(out=ab2[:, :], in0=aa[:, :], in1=b2[:, :],
                                        op=ALU.mult)
                nc.vector.tensor_scalar_mul(out=rstep[:, :], in0=rng2[:, :],
                                            scalar1=0.125)

                kn = pers.tile([128, 7], dt, tag="kn")
                for j in range(1, 8):
                    nc.vector.scalar_tensor_tensor(
                        out=kn[:, j - 1:j], in0=rstep[:, :], scalar=-float(j),
                        in1=gmin2[:, :], op0=ALU.mult, op1=ALU.subtract)
                kp = pers.tile([128, 4], dt, tag="kp")
                for q in range(4):
                    nc.vector.tensor_copy(out=kp[0:IN, q:q + 1],
                                          in_=kn[0:IN, 2 * q:2 * q + 1])
                    if 2 * q + 1 < 7:
                        nc.vector.tensor_copy(out=kp[IN:128, q:q + 1],
                                              in_=kn[IN:128, 2 * q + 1:2 * q + 2])

                cd = pers.tile([128, 4 * OUT], dt, tag="cd")
                tmp = pers.tile([128, OUT], dt, tag="cdtmp")
                P0, P1 = pwt[:, 0:32], pwt[:, 32:64]
                P2, P3 = pwt[:, 64:96], pwt[:, 96:128]
                cd0, cd1 = cd[:, 0:32], cd[:, 32:64]
                cd2, cd3 = cd[:, 64:96], cd[:, 96:128]
                nc.vector.tensor_scalar(out=cd0, in0=P1, scalar1=aa[:, 0:1],
                                        scalar2=None, op0=ALU.mult)
                nc.vector.tensor_tensor(out=cd0, in0=cd0, in1=P0, op=ALU.add)
                nc.vector.tensor_scalar(out=tmp[:, :], in0=P2, scalar1=a2[:, 0:1],
                                        scalar2=None, op0=ALU.mult)
                nc.vector.tensor_tensor(out=cd0, in0=cd0, in1=tmp[:, :], op=ALU.add)
                nc.vector.tensor_scalar(out=tmp[:, :], in0=P3, scalar1=a3[:, 0:1],
                                        scalar2=None, op0=ALU.mult)
                nc.vector.tensor_tensor(out=cd0, in0=cd0, in1=tmp[:, :], op=ALU.add)
                nc.vector.tensor_scalar(out=cd1, in0=P1, scalar1=bb[:, 0:1],
                                        scalar2=None, op0=ALU.mult)
                nc.vector.tensor_scalar(out=tmp[:, :], in0=P2, scalar1=ab[:, 0:1],
                                        scalar2=2.0, op0=ALU.mult, op1=ALU.mult)
                nc.vector.tensor_tensor(out=cd1, in0=cd1, in1=tmp[:, :], op=ALU.add)
                nc.vector.tensor_scalar(out=tmp[:, :], in0=P3, scalar1=a2b[:, 0:1],
                                        scalar2=3.0, op0=ALU.mult, op1=ALU.mult)
                nc.vector.tensor_tensor(out=cd1, in0=cd1, in1=tmp[:, :], op=ALU.add)
                nc.vector.tensor_scalar(out=cd2, in0=P2, scalar1=b2[:, 0:1],
                                        scalar2=None, op0=ALU.mult)
                nc.vector.tensor_scalar(out=tmp[:, :], in0=P3, scalar1=ab2[:, 0:1],
                                        scalar2=3.0, op0=ALU.mult, op1=ALU.mult)
                nc.vector.tensor_tensor(out=cd2, in0=cd2, in1=tmp[:, :], op=ALU.add)
                nc.vector.tensor_scalar(out=cd3, in0=P3, scalar1=b3[:, 0:1],
                                        scalar2=None, op0=ALU.mult)
                nc.vector.tensor_tensor(out=cwt[0:64, 0:32], in0=cwt[0:64, 0:32],
                                        in1=cd1[0:64, :], op=ALU.add)
                nc.vector.tensor_tensor(out=cwt[64:128, 0:32],
                                        in0=cwt[64:128, 0:32],
                                        in1=cd2[64:128, :], op=ALU.add)
                nc.vector.tensor_tensor(out=cwt[0:64, 32:64], in0=cwt[0:64, 32:64],
                                        in1=cd3[0:64, :], op=ALU.add)
                cot2 = pers.tile([IN, OUT], dt, tag="cot2")
                nc.vector.tensor_tensor(out=cot2[:, :], in0=cot[:, :],
                                        in1=cd0[0:64, :], op=ALU.add)
                bp = ps.tile([OUT, 1], dt, tag="tp")
                nc.tensor.matmul(bp[:, :], cot2[:, :], ones64[:, :])
                biasv = pers.tile([OUT, 1], dt, tag="biasv")
                nc.vector.tensor_copy(out=biasv[:, :], in_=bp[:, :])
                for j in range(1, 8):
                    q, half = (j - 1) // 2, (j - 1) % 2
                    r0, r1 = half * 64, (half + 1) * 64
                    dst = cwt[r0:r1, 32 * (12 + q):32 * (13 + q)]
                    nc.vector.tensor_scalar(out=dst,
                                            in0=rwt[r0:r1, 32 * (j - 1):32 * j],
                                            scalar1=b3[r0:r1, 0:1], scalar2=None,
                                            op0=ALU.mult)
                if debug:
                    nc.sync.dma_start(out=dbg["d_gm"][:, :], in_=gm[:, :])
                    nc.sync.dma_start(out=dbg["d_vecs"][:, :], in_=v[:, :])
                    nc.sync.dma_start(out=dbg["d_kn"][:, :], in_=kn[:, :])
                    nc.sync.dma_start(out=dbg["d_kp"][:, :], in_=kp[:, :])
                    nc.sync.dma_start(out=dbg["d_cw"][:, :], in_=cwt[:, :].bitcast(dt))
                    nc.sync.dma_start(out=dbg["d_biasv"][:, :], in_=biasv[:, :])

                # ------------ phase D: wavelet/mono/rho + round-2 -----------
                for s in range(NSUP):
                    xsl = xds[s][:, :]
                    xtop = xds[s][0:IN, :]
                    acc_a = psacc.tile([OUT, BC], dt, tag="acc2")
                    acc_b = psacc.tile([OUT, BC], dt, tag="acc2")

                    def mm2(lhs, F, first=False, last=False, rows=128):
                        nc.tensor.matmul(acc_a[:, :], f32r(lhs),
                                         f32r(F[0:rows, 0:BC]),
                                         start=first, stop=last)
                        nc.tensor.matmul(acc_b[:, :], f32r(lhs),
                                         f32r(F[0:rows, BC:SBC]),
                                         start=first, stop=last)

                    m0 = sb.tile([128, SBC], dtr, tag="m0")
                    nc.vector.tensor_copy(out=m0[0:IN, :], in_=xtop)
                    nc.scalar.square(m0[IN:128, :], xsl[IN:128, :])
                    s0 = sb.tile([IN, SBC], dt, tag="s0")
                    nc.scalar.square(s0[:, :], xtop)
                    m1 = sb.tile([128, SBC], dtr, tag="m1")
                    nc.vector.tensor_tensor(out=m1[0:IN, :], in0=s0[:, :],
                                            in1=xtop, op=ALU.mult)
                    nc.scalar.square(m1[IN:128, :], m0[IN:128, :])
                    if debug and s == 0:
                        nc.sync.dma_start(out=dbg["d_m0"][:, :], in_=m0[:, 0:256].bitcast(dt))
                        nc.sync.dma_start(out=dbg["d_m1"][:, :], in_=m1[:, 0:256].bitcast(dt))
                    mm2(cwt[:, 0:32], m0, first=True)
                    mm2(cwt[:, 32:64], m1)

                    for p in range(2):
                        u2 = sb.tile([128, SBC], dt, tag="u2")
                        nc.scalar.activation(u2[:, :], xsl, AF.Square,
                                             bias=wvt[:, 2 * p + 1:2 * p + 2],
                                             scale=wvt[:, 2 * p:2 * p + 1])
                        ew = sb.tile([128, SBC], dt, tag="ew")
                        nc.scalar.activation(ew[:, :], u2[:, :], AF.Exp,
                                             scale=-0.5)
                        wf = sb.tile([128, SBC], dtr, tag="wf")
                        nc.vector.scalar_tensor_tensor(
                            out=wf[:, :], in0=u2[:, :], scalar=1.0,
                            in1=ew[:, :], op0=ALU.subtract, op1=ALU.mult)
                        if debug and s == 0 and p == 0:
                            nc.sync.dma_start(out=dbg["d_wf"][:, :],
                                              in_=wf[:, 0:256].bitcast(dt))
                        mm2(cwt[:, 32 * (10 + p):32 * (11 + p)], wf)

                    for q in range(4):
                        rows = 128 if q < 3 else 64
                        rr = sb.tile([128, SBC], dt, tag="rr")
                        nc.gpsimd.tensor_scalar(out=rr[0:rows, :],
                                                in0=xsl[0:rows, :],
                                                scalar1=kp[0:rows, q:q + 1],
                                                scalar2=0.0, op0=ALU.add,
                                                op1=ALU.max)
                        r2 = sb.tile([128, SBC], dt, tag="r2")
                        nc.scalar.square(r2[0:rows, :], rr[0:rows, :])
                        r3 = sb.tile([128, SBC], dtr, tag="r3")
                        nc.vector.tensor_tensor(out=r3[0:rows, :],
                                                in0=r2[0:rows, :],
                                                in1=rr[0:rows, :], op=ALU.mult)
                        if debug and s == 0 and q == 0:
                            nc.sync.dma_start(out=dbg["d_r3"][:, :],
                                              in_=r3[:, 0:256].bitcast(dt))
                        mm2(cwt[0:rows, 32 * (12 + q):32 * (13 + q)], r3,
                            last=(q == 3), rows=rows)

                    yt = sb.tile([OUT, SBC], dt, tag="yt")
                    nc.vector.scalar_tensor_tensor(
                        out=yt[:, 0:BC], in0=acc_a[:, :], scalar=biasv[:, 0:1],
                        in1=y1s[s][:, 0:BC], op0=ALU.add, op1=ALU.add)
                    nc.vector.scalar_tensor_tensor(
                        out=yt[:, BC:SBC], in0=acc_b[:, :], scalar=biasv[:, 0:1],
                        in1=y1s[s][:, BC:SBC], op0=ALU.add, op1=ALU.add)
                    nc.sync.dma_start(out=y_d[:, s * SBC:(s + 1) * SBC],
                                      in_=yt[:, :])
    nc.compile()
    return nc


_NC_CACHE = None


def _get_nc():
    global _NC_CACHE
    if _NC_CACHE is None:
        _NC_CACHE = build_nc()
    return _NC_CACHE


def make_in_maps(inputs):
    consts = fold_constants(inputs)
    x = np.ascontiguousarray(np.asarray(inputs["x"], F32))
    in_maps = []
    for c in range(N_CORES):
        m = {"xs": x[c * BS:(c + 1) * BS]}
        m.update(consts)
        in_maps.append(m)
    return in_maps


def kernel(**inputs) -> np.ndarray:
    from concourse.bass_utils import run_bass_kernel_spmd
    nc = _get_nc()
    in_maps = make_in_maps(inputs)
    res = run_bass_kernel_spmd(nc, in_maps, core_ids=list(range(N_CORES)))
    out = np.concatenate([res.results[c]["y"].T for c in range(N_CORES)], axis=0)
    return np.ascontiguousarray(out, dtype=F32)

